# revision 18
# baseline (speedup 1.0000x reference)
import sys
if "/opt/trn_rl_repo" not in sys.path:
    sys.path.insert(0, "/opt/trn_rl_repo")
import math
import os
import numpy as np

# Problem dims (hardcoded per contract)
B, N, DS, DP, H, DH, DFF = 1, 1024, 768, 128, 16, 48, 1536
NC_ = 8          # cores
NLOC = N // NC_  # 128 rows per core
KT = DS // 128   # 6 k-tiles over DS
JT = N // 128    # 8 j-tiles
FKT = DFF // 128  # 12 k-tiles over DFF
SCALE = 1.0 / math.sqrt(DH)
EPS = 1e-5
VH = DH + 1      # v cols per head incl mask col

PHASE = int(os.environ.get("KPHASE", "9"))
KSUB = int(os.environ.get("KSUB", "0"))
_CACHE = {}


def _build():
    import concourse.bass as bass
    import concourse.tile as tile
    from concourse import bacc, mybir
    from concourse.masks import make_identity
    from contextlib import ExitStack

    f32 = mybir.dt.float32
    f16 = mybir.dt.float16
    AX = mybir.AxisListType
    ALU = mybir.AluOpType
    ACTF = mybir.ActivationFunctionType

    nc = bacc.Bacc("TRN2", target_bir_lowering=False, debug=False,
                   num_devices=NC_)

    def din(name, shape):
        return nc.dram_tensor(name, shape, f32, kind="ExternalInput").ap()

    def din16(name, shape):
        return nc.dram_tensor(name, shape, f16, kind="ExternalInput").ap()

    x_full = din16("x_full", [N, DS])
    s_full = din16("s_full", [N, DS])
    x_loc = din("x_loc", [NLOC, DS])
    s_loc = din16("s_loc", [NLOC, DS])
    f8 = __import__('concourse.mybir', fromlist=['mybir']).dt.float8e4
    pairT = nc.dram_tensor("pairT", [DP, N * NLOC], f8,
                           kind="ExternalInput").ap()  # [p, j, i] fp8
    mask01 = din("mask01", [N, 1])
    agw = din16("attn_gamma_w", [DS, DS]); agb = din16("attn_gamma_b", [1, DS])
    abw = din16("attn_beta_w", [DS, DS])
    wq = din16("wq", [DS, DS]); bq = din("bq", [1, DS])
    wk = din16("wk", [DS, DS]); wv = din16("wv", [DS, DS])
    wb = din16("wb", [DP, H])
    wg = din16("wg", [DS, DS]); wo = din16("wo", [DS, DS])
    atgw = din16("attn_gate_w", [DS, DS]); atgb = din16("attn_gate_b", [1, DS])
    fgw = din16("ff_gamma_w", [DS, DS]); fgb = din16("ff_gamma_b", [1, DS])
    fbw = din16("ff_beta_w", [DS, DS])
    fw1 = din16("ff_w1", [DS, DFF]); fw2 = din16("ff_w2", [DS, DFF])
    fw3 = din16("ff_w3", [DFF, DS])
    ffgw = din16("ff_gate_w", [DS, DS]); ffgb = din16("ff_gate_b", [1, DS])
    out_ap = nc.dram_tensor("out", [NLOC, DS], f32, kind="ExternalOutput").ap()

    with tile.TileContext(nc) as tc, ExitStack() as ctx:
        # ---------------- pools ----------------
        cst = ctx.enter_context(tc.tile_pool(name="cst", bufs=1))
        pers = ctx.enter_context(tc.tile_pool(name="pers", bufs=1))
        stt = ctx.enter_context(tc.tile_pool(name="stt", bufs=2))
        str16 = ctx.enter_context(tc.tile_pool(name="str16", bufs=2))
        scr = ctx.enter_context(tc.tile_pool(name="scr", bufs=2))
        ps_mm = ctx.enter_context(tc.tile_pool(name="ps_mm", bufs=2, space="PSUM"))
        ps_big = ctx.enter_context(tc.tile_pool(name="ps_big", bufs=2, space="PSUM"))

        # ---------------- constants ----------------
        ident = cst.tile([128, 128], f16)
        make_identity(nc, ident)
        ones_row = cst.tile([1, 128], f16)
        nc.vector.memset(ones_row[:], 1.0)
        ones_col = cst.tile([128, 1], f16)
        nc.vector.memset(ones_col[:], 1.0)
        ones2 = cst.tile([128, 2], f16)
        nc.vector.memset(ones2[:], 1.0)

        # wbc_aug: column-centered wb (f16) + trailing ones column
        wb16 = cst.tile([128, H], f16)
        nc.gpsimd.dma_start(wb16[:], wb[:, :])
        ps_cm = ps_mm.tile([128, 384], f32, tag="mm384")
        nc.tensor.matmul(ps_cm[0:1, 0:H], ones_col[:], wb16[:], start=True, stop=True)
        cm16 = cst.tile([1, H], f16)
        nc.scalar.activation(cm16[:], ps_cm[0:1, 0:H], ACTF.Copy, scale=1.0 / DP)
        ps_bc = ps_mm.tile([128, 384], f32, tag="mm384")
        nc.tensor.matmul(ps_bc[:, 0:H], ones_row[:], cm16[:], start=True, stop=True)
        wbc_aug = cst.tile([128, H + 1], f16)
        nc.vector.scalar_tensor_tensor(wbc_aug[:, 0:H], wb16[:], 1.0,
                                       ps_bc[:, 0:H], ALU.mult, ALU.subtract)
        nc.vector.memset(wbc_aug[:, H:H + 1], 1.0)

        # bq per-head layout [48, H]; pre-scaled; duplicated at partition base 64
        bqs = cst.tile([128, H], f32)
        nc.vector.memset(bqs[:], 0.0)
        nc.sync.dma_start(bqs[0:DH, :],
                          bq.rearrange("o (h d) -> d (o h)", d=DH))
        nc.sync.dma_start(bqs[64:64 + DH, :],
                          bq.rearrange("o (h d) -> d (o h)", d=DH))
        nc.vector.tensor_scalar(out=bqs[:], in0=bqs[:], scalar1=SCALE, scalar2=None,
                                op0=ALU.mult)

        eps_b = cst.tile([128, 1], f32)
        nc.vector.memset(eps_b[:], EPS)
        mask_sb = cst.tile([128, JT], f32)
        nc.sync.dma_start(mask_sb[:], mask01.rearrange("(t p) o -> p (t o)", p=128))

        def brow(ap_, nm):
            t = cst.tile([1, DS], f16, name=nm, tag=nm)
            nc.gpsimd.dma_start(t[:], ap_[:, :])
            return t
        agb16 = brow(agb, "agb16")
        atgb16 = brow(atgb, "atgb16")
        fgb16 = brow(fgb, "fgb16")
        ffgb16 = brow(ffgb, "ffgb16")

        # ---------------- long-lived late pools (created early: LIFO) -----
        wlate = ctx.enter_context(tc.tile_pool(name="wlate", bufs=1))
        biasd = ctx.enter_context(tc.tile_pool(name="biasd", bufs=1, space="DRAM"))
        CB = H  # bias cols per i: 16 head biases (stats kept separately)
        bias_dram = [biasd.tile([128, NLOC * CB], f16, tag=f"bd{j}",
                                name=f"bias_dram{j}") for j in range(JT)]

        # ---------------- persistent activations ----------------
        ctx2 = ExitStack()  # closed after pass A to free snT/aT space
        pAT = ctx2.enter_context(tc.tile_pool(name="pAT", bufs=1))
        snT = pAT.tile([128, KT * N], f16)
        aT = pAT.tile([128, KT * N], f16)
        kT = pers.tile([128, H * N], f16)           # per-head blocks, parts 0-47
        v_sb = pers.tile([128, JT * (H * VH)], f16)  # per (jt, h): 48 v cols + mask
        qT = pers.tile([128, H * NLOC], f16)
        snTl = pers.tile([128, KT * 128], f16)
        sTl = pers.tile([128, KT * 128], f16)
        ln_xl = pers.tile([128, DS], f16)
        g16 = pers.tile([128, DS], f16)
        gate_a = pers.tile([128, DS], f16)
        gate_f = pers.tile([128, DS], f16)
        y_part = pers.tile([128, DS], f32)

        # ---------------- helpers ----------------
        def load_w(pool, ap_, kt, n, tag, eng=None):
            t = pool.tile([128, kt * n], f16, tag=tag)
            (eng or nc.gpsimd).dma_start(
                t[:].rearrange("p (k n) -> p k n", k=kt),
                ap_.rearrange("(k p) n -> p k n", p=128))
            return t

        def transpose_768(src16, dst_sb, dst_cols):
            nkt = src16.shape[-1] // 128
            k = 0
            while k < nkt:
                npack = min(4, nkt - k)
                pk = ps_big.tile([128, 512], f16, tag="big")
                for j in range(npack):
                    nc.tensor.transpose(pk[:, j * 128:(j + 1) * 128],
                                        src16[:, (k + j) * 128:(k + j + 1) * 128],
                                        ident[:])
                for j in range(npack):
                    if j % 2 == 0:
                        nc.vector.tensor_copy(
                            dst_sb[:, dst_cols[k + j]:dst_cols[k + j] + 128],
                            pk[:, j * 128:(j + 1) * 128])
                    else:
                        nc.scalar.activation(
                            dst_sb[:, dst_cols[k + j]:dst_cols[k + j] + 128],
                            pk[:, j * 128:(j + 1) * 128], ACTF.Copy)
                k += npack

        # staged layernorm: batches scalar-engine work by activation function
        # to avoid ACT table thrash; vector does sums/stats, scalar does
        # Square (stage A), Sqrt (stage B), Identity-normalize (stage C).
        def ln_staged(xs_pool, srcs, pfx):
            nT = len(srcs)
            xts, sms, sqs, rstds, nmrs, lnts, stds = [], [], [], [], [], [], []
            for t in range(nT):
                xt = xs_pool.tile([128, DS], f16, tag=f"xin{t}")
                eng = nc.sync if srcs[t].dtype == f16 else nc.gpsimd
                eng.dma_start(xt[:], srcs[t])
                xts.append(xt)
            for t in range(nT):
                sm = stt.tile([128, 1], f32, tag=f"{pfx}sm{t}")
                nc.vector.tensor_reduce(sm[:], xts[t][:], AX.X, ALU.add)
                sms.append(sm)
                sq = stt.tile([128, 1], f32, tag=f"{pfx}sq{t}")
                sscr = scr.tile([128, DS], f16, tag="sc16a")
                nc.scalar.activation(sscr[:], xts[t][:], ACTF.Square,
                                     accum_out=sq[:])
                sqs.append(sq)
            for t in range(nT):
                mean = stt.tile([128, 1], f32, tag=f"{pfx}mean{t}")
                nc.vector.tensor_scalar(out=mean[:], in0=sms[t][:],
                                        scalar1=1.0 / DS, scalar2=None,
                                        op0=ALU.mult)
                msq = stt.tile([128, 1], f32, tag="msq")
                nc.vector.tensor_tensor(msq[:], mean[:], mean[:], ALU.mult)
                var = stt.tile([128, 1], f32, tag=f"{pfx}var{t}")
                nc.vector.scalar_tensor_tensor(var[:], sqs[t][:], 1.0 / DS, msq[:],
                                               ALU.mult, ALU.subtract)
                stds.append((mean, var))
            for t in range(nT):
                mean, var = stds[t]
                std = stt.tile([128, 1], f32, tag=f"{pfx}std{t}")
                nc.scalar.activation(std[:], var[:], ACTF.Sqrt, bias=eps_b[:])
                rstd = stt.tile([128, 1], f32, tag=f"{pfx}rstd{t}")
                nc.vector.reciprocal(rstd[:], std[:])
                rstds.append(rstd)
                nmr = stt.tile([128, 1], f32, tag=f"{pfx}nmr{t}")
                nc.vector.scalar_tensor_tensor(nmr[:], mean[:], -1.0, rstd[:],
                                               ALU.mult, ALU.mult)
                nmrs.append(nmr)
            for t in range(nT):
                lnt = xs_pool.tile([128, DS], f16, tag=f"ln{t}")
                nc.vector.scalar_tensor_tensor(
                    lnt[:], xts[t][:], rstds[t][:],
                    nmrs[t][:].broadcast_to([128, DS]),
                    ALU.mult, ALU.add)
                lnts.append(lnt)
            return lnts

        def modulate(lhs_fn, gamma16, gbias16, beta16, lnx16):
            """a = sigmoid(sn@gamma + gb) * lnx + sn@beta; lhs_fn(kt)->snT ktile AP"""
            pg = [ps_mm.tile([128, 384], f32, tag="mm384", name=f"pg{_i}") for _i in range(2)]
            pb = [ps_mm.tile([128, 384], f32, tag="mm384", name=f"pb{_i}") for _i in range(2)]
            for kt in range(KT):
                lhs = lhs_fn(kt)
                for c in range(2):
                    nc.tensor.matmul(
                        pg[c][:], lhs,
                        gamma16[:, kt * DS + c * 384:kt * DS + (c + 1) * 384],
                        start=(kt == 0), stop=False)
                    nc.tensor.matmul(
                        pb[c][:], lhs,
                        beta16[:, kt * DS + c * 384:kt * DS + (c + 1) * 384],
                        start=(kt == 0), stop=(kt == KT - 1))
            for c in range(2):
                nc.tensor.matmul(pg[c][:], ones_row[:],
                                 gbias16[:, c * 384:(c + 1) * 384],
                                 start=False, stop=True)
            sig = scr.tile([128, DS], f16, tag="sig")
            for c in range(2):
                nc.scalar.activation(sig[:, c * 384:(c + 1) * 384], pg[c][:],
                                     ACTF.Sigmoid)
            tmp = scr.tile([128, DS], f16, tag="sigl")
            nc.vector.tensor_tensor(tmp[:], sig[:], lnx16[:], ALU.mult)
            a16 = str16.tile([128, DS], f16, tag="am")
            for c in range(2):
                nc.vector.scalar_tensor_tensor(
                    a16[:, c * 384:(c + 1) * 384],
                    tmp[:, c * 384:(c + 1) * 384], 1.0, pb[c][:],
                    ALU.mult, ALU.add)
            return a16

        def gate_from(sT_sb, gatew16, gbias16, dst16):
            p = [ps_mm.tile([128, 384], f32, tag="mm384", name=f"pgate{_i}") for _i in range(2)]
            for kt in range(KT):
                lhs = sT_sb[:, kt * 128:(kt + 1) * 128]
                for c in range(2):
                    nc.tensor.matmul(
                        p[c][:], lhs,
                        gatew16[:, kt * DS + c * 384:kt * DS + (c + 1) * 384],
                        start=(kt == 0), stop=False)
            for c in range(2):
                nc.tensor.matmul(p[c][:], ones_row[:],
                                 gbias16[:, c * 384:(c + 1) * 384],
                                 start=False, stop=True)
            for c in range(2):
                nc.scalar.activation(dst16[:, c * 384:(c + 1) * 384], p[c][:],
                                     ACTF.Sigmoid)

        # ================= phase 1: LN + modulation =================
        x3 = x_full.rearrange("(t p) d -> p t d", p=128)
        s3 = s_full.rearrange("(t p) d -> p t d", p=128)

        with tc.tile_pool(name="wmod", bufs=1) as wmod, \
             tc.tile_pool(name="xs", bufs=1) as xs:
            agw16 = load_w(wmod, agw, KT, DS, "agw")
            abw16 = load_w(wmod, abw, KT, DS, "abw")

            # staged LN for s (8 full tiles + local)
            s_srcs = [s3[:, t, :] for t in range(JT)] + [s_loc[:, :]]
            lns_all = ln_staged(xs, s_srcs, "s")
            for t in range(JT):
                transpose_768(lns_all[t], snT, [kt * N + t * 128 for kt in range(KT)])
            transpose_768(lns_all[JT], snTl, [kt * 128 for kt in range(KT)])
            # raw s transposed (for gates)
            s16l = str16.tile([128, DS], f16, tag="ln")
            nc.gpsimd.dma_start(s16l[:], s_loc[:, :])
            transpose_768(s16l, sTl, [kt * 128 for kt in range(KT)])

            # staged LN for x (8 full tiles + local; x_loc is f32, dma converts)
            x_srcs = [x3[:, t, :] for t in range(JT)] + [x_loc[:, :]]
            lnx_all = ln_staged(xs, x_srcs, "x")
            nc.vector.tensor_copy(ln_xl[:], lnx_all[JT][:])

            # modulate stretch: scalar does only sigmoids here
            for t in range(JT):
                a16 = modulate(
                    lambda kt: snT[:, kt * N + t * 128:kt * N + (t + 1) * 128],
                    agw16, agb16, abw16, lnx_all[t])
                transpose_768(a16, aT, [kt * N + t * 128 for kt in range(KT)])
            al16 = modulate(
                lambda kt: snTl[:, kt * 128:(kt + 1) * 128],
                agw16, agb16, abw16, ln_xl)
            aTl = pers.tile([128, KT * 128], f16)
            transpose_768(al16, aTl, [kt * 128 for kt in range(KT)])

        def emit_debug(src_t):
            w = min(src_t.shape[-1], DS)
            dbg = scr.tile([128, DS], f32, tag="dbg")
            nc.vector.memset(dbg[:], 0.0)
            nc.vector.tensor_copy(dbg[:, 0:w], src_t[:, 0:w])
            nc.sync.dma_start(out_ap[:, :], dbg[:])

        if PHASE == 1:
            emit_debug(al16)
            ctx2.close()

        if PHASE >= 2:
            # ====== pair prefetch (consumed in phase 3; sync queue only) ======
            ctx3 = ExitStack()
            pairp = ctx3.enter_context(tc.tile_pool(name="pairp", bufs=4))
            pair_tiles = {}

            def fetch_pair(c):
                Tt = pairp.tile([128, NLOC * 64], f8, tag="pairT",
                                name=f"pt{c}")
                eng = nc.sync if c % 2 == 0 else nc.scalar
                eng.dma_start(
                    Tt[:], pairT[:, c * NLOC * 64:(c + 1) * NLOC * 64])
                pair_tiles[c] = Tt

            # prefetch first 4 chunks now (2 DMA queues); rest emitted in
            # pass A as buffers free up, to keep queue streams deadlock-free
            for c in range(4):
                fetch_pair(c)

            # ================= phase 2: k/v/q/g + gates =================
            with tc.tile_pool(name="wkp", bufs=1) as wkp:
                wk16 = load_w(wkp, wk, KT, DS, "wk")

                # kT: per head, partitions 0-47
                for h in range(H):
                    for half in range(2):
                        pk = ps_big.tile([128, 512], f32, tag="big")
                        for kt in range(KT):
                            rhs = aT[:, kt * N + half * 512:kt * N + (half + 1) * 512]
                            nc.tensor.matmul(
                                pk[0:DH, :],
                                wk16[:, kt * DS + h * DH:kt * DS + (h + 1) * DH],
                                rhs, start=(kt == 0), stop=(kt == KT - 1))
                        nc.vector.tensor_copy(
                            kT[0:DH, h * N + half * 512:h * N + (half + 1) * 512],
                            pk[0:DH, :])

            with tc.tile_pool(name="wvp", bufs=1) as wvp:
                wv16 = load_w(wvp, wv, KT, DS, "wv")
                # v rows (masked), per (jt, head): 48 v cols + mask col
                for t in range(JT):
                    pv = [ps_mm.tile([128, 384], f32, tag="mm384", name=f"pv{_i}") for _i in range(2)]
                    for kt in range(KT):
                        lhs = aT[:, kt * N + t * 128:kt * N + (t + 1) * 128]
                        for c in range(2):
                            nc.tensor.matmul(
                                pv[c][:], lhs,
                                wv16[:, kt * DS + c * 384:kt * DS + (c + 1) * 384],
                                start=(kt == 0), stop=(kt == KT - 1))
                    for c in range(2):
                        dst = v_sb[:, t * H * VH + c * 8 * VH:
                                   t * H * VH + (c + 1) * 8 * VH]
                        nc.scalar.activation(
                            dst.rearrange("p (h v) -> p h v", v=VH)[:, :, 0:DH],
                            pv[c][:].rearrange("p (h d) -> p h d", d=DH),
                            ACTF.Copy, scale=mask_sb[:, t:t + 1])
                    mdst = v_sb[:, t * H * VH:(t + 1) * H * VH]
                    nc.vector.tensor_copy(
                        mdst.rearrange("p (h v) -> p h v", v=VH)[:, :, DH:VH],
                        mask_sb[:, t:t + 1, None].broadcast_to([128, H, 1]))

            with tc.tile_pool(name="wqp", bufs=1) as wqp:
                wq16 = load_w(wqp, wq, KT, DS, "wq")
                # qT local (scaled by 1/sqrt(dh), + bq): per head, base 0
                for hq in range(0, H, 4):
                    pq = ps_big.tile([128, 512], f32, tag="big")
                    for hh in range(4):
                        h = hq + hh
                        for kt in range(KT):
                            rhs = aTl[:, kt * 128:(kt + 1) * 128]
                            nc.tensor.matmul(
                                pq[0:DH, hh * 128:(hh + 1) * 128],
                                wq16[:, kt * DS + h * DH:kt * DS + (h + 1) * DH],
                                rhs, start=(kt == 0 and hh == 0),
                                stop=(kt == KT - 1 and hh == 3),
                                skip_group_check=True)
                        nc.scalar.activation(
                            qT[0:DH, h * 128:(h + 1) * 128],
                            pq[0:DH, hh * 128:(hh + 1) * 128],
                            ACTF.Identity, bias=bqs[0:DH, h:h + 1], scale=SCALE)

            with tc.tile_pool(name="wgg", bufs=1) as wgg:
                wg16 = load_w(wgg, wg, KT, DS, "wg")
                atgw16 = load_w(wgg, atgw, KT, DS, "atgw")
                ffgw16 = load_w(wgg, ffgw, KT, DS, "ffgw")

                # g = sigmoid(a_loc @ wg)
                pgt = [ps_mm.tile([128, 384], f32, tag="mm384", name=f"pgt{_i}") for _i in range(2)]
                for kt in range(KT):
                    lhs = aTl[:, kt * 128:(kt + 1) * 128]
                    for c in range(2):
                        nc.tensor.matmul(
                            pgt[c][:], lhs,
                            wg16[:, kt * DS + c * 384:kt * DS + (c + 1) * 384],
                            start=(kt == 0), stop=(kt == KT - 1))
                for c in range(2):
                    nc.scalar.activation(g16[:, c * 384:(c + 1) * 384], pgt[c][:],
                                         ACTF.Sigmoid)
                gate_from(sTl, atgw16, atgb16, gate_a)
                gate_from(sTl, ffgw16, ffgb16, gate_f)
            if PHASE == 2:
                emit_debug(g16)

            # ================= phase 3: pair bias + attention =================
            if PHASE >= 3:
                QI = 32  # i-rows per square quarter-tile
                sqp = ctx3.enter_context(tc.tile_pool(name="sqp", bufs=2))
                bstg = ctx3.enter_context(tc.tile_pool(name="bstg", bufs=2))
                pstat = ctx3.enter_context(tc.tile_pool(name="pstat", bufs=2))
                ps_bias = ctx3.enter_context(
                    tc.tile_pool(name="ps_bias", bufs=2, space="PSUM"))
                # prefetch wo during pass A (gpsimd queue; pool made early)
                wo16 = load_w(wlate, wo, KT, DS, "wo")

                # ---- pass A: pair-cond LN + per-head bias, spilled to DRAM ----
                # pbias layout per 16-i block: [bi*16 bias][256+bi meansum]
                # [272+bi sumsq] -- stats land contiguous per block, so the
                # epilogue runs on dense tiles (strided extracts are ~50x slow)
                ones1 = wbc_aug[:, H:H + 1]
                for jt in range(JT):
                    bias_jt = bstg.tile([128, NLOC * CB], f16, tag="bstg")
                    stat_jt = bstg.tile([128, 2 * NLOC], f32, tag="sstg")
                    for ih in range(2):
                        Tt = pair_tiles[jt * 2 + ih]
                        Tv = Tt[:].rearrange("p (i j) -> p i j", j=128)
                        for hb in range(NLOC // 32):
                            if hb % (QI // 16) == 0:
                                q0 = hb * 16
                                SQ = sqp.tile([128, QI * 128], f16, tag="sq")
                                nc.vector.tensor_tensor(
                                    SQ[:],
                                    Tt[:, q0 * 128:(q0 + QI) * 128],
                                    Tt[:, q0 * 128:(q0 + QI) * 128], ALU.mult)
                                SQv = SQ[:].rearrange("p (i j) -> p i j", j=128)
                            pbias = ps_bias.tile([128, 16 * H + 33], f32, tag="pb")
                            for bi in range(16):
                                i = hb * 16 + bi
                                nc.tensor.matmul(
                                    pbias[:, bi * H:(bi + 1) * H],
                                    Tv[:, i, :], wbc_aug[:, 0:H],
                                    start=True, stop=True, skip_group_check=True)
                                nc.tensor.matmul(
                                    pbias[:, 16 * H + bi:16 * H + bi + 1],
                                    Tv[:, i, :], ones1,
                                    start=True, stop=True, skip_group_check=True)
                                nc.tensor.matmul(
                                    pbias[:, 16 * H + 16 + bi:16 * H + 17 + bi],
                                    SQv[:, i - q0, :], ones1,
                                    start=True, stop=True, skip_group_check=True)
                            i0 = ih * 64 + hb * 16
                            nc.scalar.activation(
                                bias_jt[:, i0 * CB:(i0 + 16) * CB],
                                pbias[:, 0:16 * H], ACTF.Copy)
                            nc.scalar.activation(
                                stat_jt[:, i0:i0 + 16],
                                pbias[:, 16 * H:16 * H + 16], ACTF.Copy)
                            nc.scalar.activation(
                                stat_jt[:, NLOC + i0:NLOC + i0 + 16],
                                pbias[:, 16 * H + 16:16 * H + 32], ACTF.Copy)
                        c = jt * 2 + ih
                        if c + 4 < JT * 2:
                            fetch_pair(c + 4)

                    # per-jt epilogue: LN stats + in-place scaling + DRAM spill
                    mean = pstat.tile([128, NLOC], f32, tag="mean")
                    nc.vector.tensor_scalar(
                        out=mean[:], in0=stat_jt[:, 0:NLOC],
                        scalar1=1.0 / DP, scalar2=None, op0=ALU.mult)
                    msq = pstat.tile([128, NLOC], f32, tag="msq")
                    nc.vector.tensor_tensor(msq[:], mean[:], mean[:], ALU.mult)
                    var = pstat.tile([128, NLOC], f32, tag="var")
                    nc.vector.scalar_tensor_tensor(
                        var[:], stat_jt[:, NLOC:2 * NLOC], 1.0 / DP,
                        msq[:], ALU.mult, ALU.subtract)
                    std = pstat.tile([128, NLOC], f32, tag="std")
                    nc.scalar.activation(std[:], var[:], ACTF.Sqrt, bias=eps_b[:])
                    rstd = pstat.tile([128, NLOC], f32, tag="rstd")
                    nc.vector.reciprocal(rstd[:], std[:])
                    nc.vector.scalar_tensor_tensor(
                        bias_jt[:].rearrange("p (i c) -> p i c", c=CB),
                        bias_jt[:].rearrange("p (i c) -> p i c", c=CB),
                        1.0,
                        rstd[:, :, None].broadcast_to([128, NLOC, CB]),
                        ALU.mult, ALU.mult)
                    nc.gpsimd.dma_start(bias_dram[jt][:], bias_jt[:])

                ctx3.close()  # free pair + SQ + staging SBUF space
                ctx2.close()  # free snT/aT SBUF space

                # prefetch ff weights during pass B (gpsimd queue)
                wff = ctx.enter_context(tc.tile_pool(name="wff", bufs=1))
                fgw16 = load_w(wff, fgw, KT, DS, "fgw")
                fbw16 = load_w(wff, fbw, KT, DS, "fbw")
                fw116 = load_w(wff, fw1, KT, DFF, "fw1", eng=nc.sync)
                fw216 = load_w(wff, fw2, KT, DFF, "fw2", eng=nc.sync)

                with tc.tile_pool(name="biasrd", bufs=2) as biasrd, \
                     tc.tile_pool(name="attnT", bufs=3) as attnTp, \
                     tc.tile_pool(name="ps_o", bufs=1, space="PSUM") as ps_o:

                    po = [ps_o.tile([128, 8 * VH], f32, tag=f"o{i}", name=f"po{i}") for i in range(2)]

                    # ---- pass B: attention over all j-tiles ----
                    for jt in range(JT):
                        brt = biasrd.tile([128, NLOC * CB], f16, tag="brd")
                        nc.sync.dma_start(brt[:], bias_dram[jt][:])
                        bias_hi = brt[:].rearrange("p (i h) -> p h i", h=CB)
                        for grp in range(4):
                            pl = ps_big.tile([128, 512], f32, tag="big")
                            for hh in range(4):
                                h = grp * 4 + hh
                                nc.tensor.matmul(
                                    pl[:, hh * 128:(hh + 1) * 128],
                                    kT[0:DH,
                                       h * N + jt * 128:h * N + (jt + 1) * 128],
                                    qT[0:DH, h * 128:(h + 1) * 128],
                                    start=(hh == 0), stop=False,
                                    skip_group_check=True)
                            nc.tensor.matmul(
                                pl[:], ident[:],
                                bias_hi[:, grp * 4:(grp + 1) * 4, :],
                                start=False, stop=True, skip_group_check=True)
                            at16 = attnTp.tile([128, 512], f16, tag="attnT")
                            nc.scalar.activation(at16[:], pl[:], ACTF.Exp)
                            for hh in range(4):
                                h = grp * 4 + hh
                                ho = h % 8
                                pot = po[h // 8]
                                lhs = at16[:, hh * 128:(hh + 1) * 128]
                                nc.tensor.matmul(
                                    pot[:, ho * VH:(ho + 1) * VH], lhs,
                                    v_sb[:, jt * H * VH + h * VH:
                                         jt * H * VH + (h + 1) * VH],
                                    start=(jt == 0 and ho == 0),
                                    stop=(jt == JT - 1 and ho == 7),
                                    skip_group_check=True)

                    if KSUB == 0:
                        # ---- normalize + gate + output projection ----
                        rd = stt.tile([128, H], f32, tag="rd")
                        for i in range(2):
                            den = po[i][:].rearrange("p (h d) -> p h d", d=VH)[:, :, DH]
                            nc.vector.reciprocal(rd[:, i * 8:(i + 1) * 8], den)
                        g2 = scr.tile([128, DS], f16, tag="sc16a")
                        nc.vector.tensor_tensor(
                            g2[:].rearrange("p (h d) -> p h d", d=DH),
                            g16[:].rearrange("p (h d) -> p h d", d=DH),
                            rd[:, :, None].broadcast_to([128, H, DH]), ALU.mult)
                        go = str16.tile([128, DS], f16, tag="go")
                        for i in range(2):
                            nc.vector.scalar_tensor_tensor(
                                go[:, i * 384:(i + 1) * 384].rearrange(
                                    "p (h d) -> p h d", d=DH),
                                po[i][:].rearrange("p (h d) -> p h d", d=VH)[:, :, 0:DH],
                                1.0,
                                g2[:, i * 384:(i + 1) * 384].rearrange(
                                    "p (h d) -> p h d", d=DH),
                                ALU.mult, ALU.mult)
                        goT = pers.tile([128, KT * 128], f16)
                        transpose_768(go, goT, [kt * 128 for kt in range(KT)])
                        if PHASE == 3:
                            emit_debug(go)

                if PHASE >= 4 and KSUB == 0:
                    scr4 = ctx.enter_context(tc.tile_pool(name="scr4", bufs=2))
                    pao = [ps_mm.tile([128, 384], f32, tag="mm384", name=f"pao{_i}") for _i in range(2)]
                    for kt in range(KT):
                        lhs = goT[:, kt * 128:(kt + 1) * 128]
                        for c in range(2):
                            nc.tensor.matmul(
                                pao[c][:], lhs,
                                wo16[:, kt * DS + c * 384:kt * DS + (c + 1) * 384],
                                start=(kt == 0), stop=(kt == KT - 1))
                    # y_part = x_loc + gate_a * attn_out
                    xl32 = scr4.tile([128, DS], f32, tag="s32")
                    nc.sync.dma_start(xl32[:], x_loc[:, :])
                    for c in range(2):
                        nc.vector.scalar_tensor_tensor(
                            y_part[:, c * 384:(c + 1) * 384],
                            gate_a[:, c * 384:(c + 1) * 384], 1.0, pao[c][:],
                            ALU.mult, ALU.mult)
                    nc.vector.tensor_tensor(y_part[:], y_part[:], xl32[:], ALU.add)

                    # ================= phase 4: feed-forward =================
                    # load fw3 now (gpsimd queue; needed only after m16/mT)
                    fw316 = load_w(wff, fw3, FKT, DS, "fw3")
                    f16l = modulate(
                        lambda kt: snTl[:, kt * 128:(kt + 1) * 128],
                        fgw16, fgb16, fbw16, ln_xl)
                    fT = pers.tile([128, KT * 128], f16)
                    transpose_768(f16l, fT, [kt * 128 for kt in range(KT)])

                    m16 = str16.tile([128, DFF], f16, tag="m16")
                    for c in range(4):  # 4 chunks of 384 over DFF
                        ph1 = ps_mm.tile([128, 384], f32, tag="mm384")
                        ph2 = ps_mm.tile([128, 384], f32, tag="mm384")
                        for kt in range(KT):
                            lhs = fT[:, kt * 128:(kt + 1) * 128]
                            nc.tensor.matmul(
                                ph1[:], lhs,
                                fw116[:, kt * DFF + c * 384:kt * DFF + (c + 1) * 384],
                                start=(kt == 0), stop=(kt == KT - 1))
                            nc.tensor.matmul(
                                ph2[:], lhs,
                                fw216[:, kt * DFF + c * 384:kt * DFF + (c + 1) * 384],
                                start=(kt == 0), stop=(kt == KT - 1))
                        sg = scr4.tile([128, 384], f16, tag="sil")
                        nc.scalar.activation(sg[:], ph1[:], ACTF.Sigmoid)
                        pp = scr4.tile([128, 384], f16, tag="pp")
                        nc.vector.scalar_tensor_tensor(
                            pp[:], sg[:], 1.0, ph1[:], ALU.mult, ALU.mult)
                        nc.vector.scalar_tensor_tensor(
                            m16[:, c * 384:(c + 1) * 384], pp[:], 1.0,
                            ph2[:], ALU.mult, ALU.mult)
                    mT = pers.tile([128, FKT * 128], f16)
                    transpose_768(m16, mT, [kt * 128 for kt in range(FKT)])

                    pff = [ps_mm.tile([128, 384], f32, tag="mm384", name=f"pff{_i}") for _i in range(2)]
                    for kt in range(FKT):
                        lhs = mT[:, kt * 128:(kt + 1) * 128]
                        for c in range(2):
                            nc.tensor.matmul(
                                pff[c][:], lhs,
                                fw316[:, kt * DS + c * 384:kt * DS + (c + 1) * 384],
                                start=(kt == 0), stop=(kt == FKT - 1))
                    yout = scr4.tile([128, DS], f32, tag="s32")
                    for c in range(2):
                        nc.vector.scalar_tensor_tensor(
                            yout[:, c * 384:(c + 1) * 384],
                            gate_f[:, c * 384:(c + 1) * 384], 1.0, pff[c][:],
                            ALU.mult, ALU.mult)
                    nc.vector.tensor_tensor(yout[:], yout[:], y_part[:], ALU.add)
                    nc.sync.dma_start(out_ap[:, :], yout[:])

    nc.compile()
    return nc


def _get_nc():
    if "nc" not in _CACHE:
        _CACHE["nc"] = _build()
    return _CACHE["nc"]


def prepare_in_maps(inputs):
    x = np.asarray(inputs["x"], dtype=np.float32).reshape(N, DS)
    s = np.asarray(inputs["single_cond"], dtype=np.float32).reshape(N, DS)
    pc = np.asarray(inputs["pair_cond"], dtype=np.float32).reshape(N, N, DP)
    mask = np.asarray(inputs["mask"]).reshape(N, 1).astype(np.float32)
    x16 = x.astype(np.float16)
    s16 = s.astype(np.float16)
    w16 = {k: np.asarray(inputs[k], dtype=np.float32).astype(np.float16) for k in [
        "attn_gamma_w", "attn_beta_w", "wq", "wk", "wv", "wb", "wg", "wo",
        "attn_gate_w", "ff_gamma_w", "ff_beta_w", "ff_w1", "ff_w2", "ff_w3",
        "ff_gate_w"]}
    rows16 = {k: np.asarray(inputs[k], dtype=np.float32).reshape(1, DS).astype(np.float16)
              for k in ["attn_gamma_b", "attn_gate_b", "ff_gamma_b", "ff_gate_b"]}
    bq_row = np.asarray(inputs["bq"], dtype=np.float32).reshape(1, DS)
    in_maps = []
    for c in range(NC_):
        blk = pc[c * NLOC:(c + 1) * NLOC]          # [i_loc, j, p]
        from concourse import mybir as _mb
        f8np = _mb.dt.np(_mb.dt.float8e4)
        pairTm = np.empty((DP, JT, NLOC, 128), f8np)
        bt = blk.transpose(2, 0, 1)                # [p, i_loc, j] view
        for jt in range(JT):
            pairTm[:, jt] = bt[:, :, jt * 128:(jt + 1) * 128]
        m = {"x_full": x16, "s_full": s16,
             "x_loc": x[c * NLOC:(c + 1) * NLOC],
             "s_loc": s16[c * NLOC:(c + 1) * NLOC],
             "pairT": pairTm.reshape(DP, N * NLOC),
             "mask01": mask,
             "bq": bq_row}
        m.update(w16)
        m.update(rows16)
        in_maps.append(m)
    return in_maps


def kernel(**inputs):
    from concourse.bass_utils import run_bass_kernel_spmd
    nc = _get_nc()
    in_maps = prepare_in_maps(inputs)
    _CACHE["in_maps"] = in_maps
    globals()["_last_in_maps"] = in_maps
    res = run_bass_kernel_spmd(nc, in_maps, list(range(NC_)))
    out = np.concatenate([res.results[c]["out"] for c in range(NC_)], axis=0)
    return out.reshape(B, N, DS).astype(np.float32)


# revision 20
# speedup vs baseline: 1.3338x; 1.3338x over previous
import sys
if "/opt/trn_rl_repo" not in sys.path:
    sys.path.insert(0, "/opt/trn_rl_repo")
import math
import os
import numpy as np

# Problem dims (hardcoded per contract)
B, N, DS, DP, H, DH, DFF = 1, 1024, 768, 128, 16, 48, 1536
NC_ = 8          # cores
NLOC = N // NC_  # 128 rows per core
KT = DS // 128   # 6 k-tiles over DS
JT = N // 128    # 8 j-tiles
FKT = DFF // 128  # 12 k-tiles over DFF
SCALE = 1.0 / math.sqrt(DH)
EPS = 1e-5
VH = DH + 1      # v cols per head incl mask col

PHASE = int(os.environ.get("KPHASE", "9"))
KSUB = int(os.environ.get("KSUB", "0"))
_CACHE = {}


def _build():
    import concourse.bass as bass
    import concourse.tile as tile
    from concourse import bacc, mybir
    from concourse.masks import make_identity
    from contextlib import ExitStack

    f32 = mybir.dt.float32
    f16 = mybir.dt.float16
    AX = mybir.AxisListType
    ALU = mybir.AluOpType
    ACTF = mybir.ActivationFunctionType

    nc = bacc.Bacc("TRN2", target_bir_lowering=False, debug=False,
                   num_devices=NC_)

    def din(name, shape):
        return nc.dram_tensor(name, shape, f32, kind="ExternalInput").ap()

    def din16(name, shape):
        return nc.dram_tensor(name, shape, f16, kind="ExternalInput").ap()

    x_full = din16("x_full", [N, DS])
    s_full = din16("s_full", [N, DS])
    x_loc = din("x_loc", [NLOC, DS])
    s_loc = din16("s_loc", [NLOC, DS])
    pairT = din16("pairT", [DP, N * NLOC])  # [p, j, i_local] f16, host-transposed
    mask01 = din("mask01", [N, 1])
    agw = din16("attn_gamma_w", [DS, DS]); agb = din16("attn_gamma_b", [1, DS])
    abw = din16("attn_beta_w", [DS, DS])
    wq = din16("wq", [DS, DS]); bq = din("bq", [1, DS])
    wk = din16("wk", [DS, DS]); wv = din16("wv", [DS, DS])
    wb = din16("wb", [DP, H])
    wg = din16("wg", [DS, DS]); wo = din16("wo", [DS, DS])
    atgw = din16("attn_gate_w", [DS, DS]); atgb = din16("attn_gate_b", [1, DS])
    fgw = din16("ff_gamma_w", [DS, DS]); fgb = din16("ff_gamma_b", [1, DS])
    fbw = din16("ff_beta_w", [DS, DS])
    fw1 = din16("ff_w1", [DS, DFF]); fw2 = din16("ff_w2", [DS, DFF])
    fw3 = din16("ff_w3", [DFF, DS])
    ffgw = din16("ff_gate_w", [DS, DS]); ffgb = din16("ff_gate_b", [1, DS])
    out_ap = nc.dram_tensor("out", [NLOC, DS], f32, kind="ExternalOutput").ap()

    with tile.TileContext(nc) as tc, ExitStack() as ctx:
        # ---------------- pools ----------------
        cst = ctx.enter_context(tc.tile_pool(name="cst", bufs=1))
        pers = ctx.enter_context(tc.tile_pool(name="pers", bufs=1))
        stt = ctx.enter_context(tc.tile_pool(name="stt", bufs=2))
        str16 = ctx.enter_context(tc.tile_pool(name="str16", bufs=2))
        scr = ctx.enter_context(tc.tile_pool(name="scr", bufs=2))
        ps_mm = ctx.enter_context(tc.tile_pool(name="ps_mm", bufs=2, space="PSUM"))
        ps_big = ctx.enter_context(tc.tile_pool(name="ps_big", bufs=2, space="PSUM"))

        # ---------------- constants ----------------
        ident = cst.tile([128, 128], f16)
        make_identity(nc, ident)
        ones_row = cst.tile([1, 128], f16)
        nc.vector.memset(ones_row[:], 1.0)
        ones_col = cst.tile([128, 1], f16)
        nc.vector.memset(ones_col[:], 1.0)
        ones2 = cst.tile([128, 2], f16)
        nc.vector.memset(ones2[:], 1.0)

        # wbc_aug: column-centered wb (f16) + trailing ones column
        wb16 = cst.tile([128, H], f16)
        nc.gpsimd.dma_start(wb16[:], wb[:, :])
        ps_cm = ps_mm.tile([128, 384], f32, tag="mm384")
        nc.tensor.matmul(ps_cm[0:1, 0:H], ones_col[:], wb16[:], start=True, stop=True)
        cm16 = cst.tile([1, H], f16)
        nc.scalar.activation(cm16[:], ps_cm[0:1, 0:H], ACTF.Copy, scale=1.0 / DP)
        ps_bc = ps_mm.tile([128, 384], f32, tag="mm384")
        nc.tensor.matmul(ps_bc[:, 0:H], ones_row[:], cm16[:], start=True, stop=True)
        wbc_aug = cst.tile([128, H + 1], f16)
        nc.vector.scalar_tensor_tensor(wbc_aug[:, 0:H], wb16[:], 1.0,
                                       ps_bc[:, 0:H], ALU.mult, ALU.subtract)
        nc.vector.memset(wbc_aug[:, H:H + 1], 1.0)

        # bq per-head layout [48, H]; pre-scaled; duplicated at partition base 64
        bqs = cst.tile([128, H], f32)
        nc.vector.memset(bqs[:], 0.0)
        nc.sync.dma_start(bqs[0:DH, :],
                          bq.rearrange("o (h d) -> d (o h)", d=DH))
        nc.sync.dma_start(bqs[64:64 + DH, :],
                          bq.rearrange("o (h d) -> d (o h)", d=DH))
        nc.vector.tensor_scalar(out=bqs[:], in0=bqs[:], scalar1=SCALE, scalar2=None,
                                op0=ALU.mult)

        eps_b = cst.tile([128, 1], f32)
        nc.vector.memset(eps_b[:], EPS)
        mask_sb = cst.tile([128, JT], f32)
        nc.sync.dma_start(mask_sb[:], mask01.rearrange("(t p) o -> p (t o)", p=128))

        def brow(ap_, nm):
            t = cst.tile([1, DS], f16, name=nm, tag=nm)
            nc.gpsimd.dma_start(t[:], ap_[:, :])
            return t
        agb16 = brow(agb, "agb16")
        atgb16 = brow(atgb, "atgb16")
        fgb16 = brow(fgb, "fgb16")
        ffgb16 = brow(ffgb, "ffgb16")

        # ---------------- long-lived late pools (created early: LIFO) -----
        wlate = ctx.enter_context(tc.tile_pool(name="wlate", bufs=1))
        biasd = ctx.enter_context(tc.tile_pool(name="biasd", bufs=1, space="DRAM"))
        CB = H  # bias cols per i: 16 head biases (stats kept separately)
        bias_dram = [biasd.tile([128, NLOC * CB], f16, tag=f"bd{j}",
                                name=f"bias_dram{j}") for j in range(JT)]

        # ---------------- persistent activations ----------------
        ctx2 = ExitStack()  # closed after pass A to free aT space
        pAT = ctx2.enter_context(tc.tile_pool(name="pAT", bufs=1))
        aT = pAT.tile([128, KT * N], f16)
        ctxsn = ExitStack()  # snT freed right after phase 1
        snp = ctxsn.enter_context(tc.tile_pool(name="snp", bufs=1))
        snT = snp.tile([128, KT * N], f16)
        kT = pers.tile([128, H * N], f16)           # per-head blocks, parts 0-47
        v_sb = pers.tile([128, JT * (H * VH)], f16)  # per (jt, h): 48 v cols + mask
        qT = pers.tile([128, H * NLOC], f16)
        snTl = pers.tile([128, KT * 128], f16)
        sTl = pers.tile([128, KT * 128], f16)
        ln_xl = pers.tile([128, DS], f16)
        g16 = pers.tile([128, DS], f16)
        gate_a = pers.tile([128, DS], f16)
        gate_f = pers.tile([128, DS], f16)
        y_part = pers.tile([128, DS], f32)

        # ---------------- helpers ----------------
        def load_w(pool, ap_, kt, n, tag, eng=None):
            t = pool.tile([128, kt * n], f16, tag=tag)
            (eng or nc.gpsimd).dma_start(
                t[:].rearrange("p (k n) -> p k n", k=kt),
                ap_.rearrange("(k p) n -> p k n", p=128))
            return t

        def transpose_768(src16, dst_sb, dst_cols):
            nkt = src16.shape[-1] // 128
            k = 0
            while k < nkt:
                npack = min(4, nkt - k)
                pk = ps_big.tile([128, 512], f16, tag="big")
                for j in range(npack):
                    nc.tensor.transpose(pk[:, j * 128:(j + 1) * 128],
                                        src16[:, (k + j) * 128:(k + j + 1) * 128],
                                        ident[:])
                for j in range(npack):
                    if j % 2 == 0:
                        nc.vector.tensor_copy(
                            dst_sb[:, dst_cols[k + j]:dst_cols[k + j] + 128],
                            pk[:, j * 128:(j + 1) * 128])
                    else:
                        nc.scalar.activation(
                            dst_sb[:, dst_cols[k + j]:dst_cols[k + j] + 128],
                            pk[:, j * 128:(j + 1) * 128], ACTF.Copy)
                k += npack

        # staged layernorm: batches scalar-engine work by activation function
        # to avoid ACT table thrash; vector does sums/stats, scalar does
        # Square (stage A), Sqrt (stage B), Identity-normalize (stage C).
        def ln_staged(xs_pool, srcs, pfx):
            nT = len(srcs)
            xts, sms, sqs, rstds, nmrs, lnts, stds = [], [], [], [], [], [], []
            for t in range(nT):
                xt = xs_pool.tile([128, DS], f16, tag=f"xin{t}")
                eng = nc.sync if srcs[t].dtype == f16 else nc.gpsimd
                eng.dma_start(xt[:], srcs[t])
                xts.append(xt)
            for t in range(nT):
                sm = stt.tile([128, 1], f32, tag=f"{pfx}sm{t}")
                nc.vector.tensor_reduce(sm[:], xts[t][:], AX.X, ALU.add)
                sms.append(sm)
                sq = stt.tile([128, 1], f32, tag=f"{pfx}sq{t}")
                sscr = scr.tile([128, DS], f16, tag="sc16a")
                nc.scalar.activation(sscr[:], xts[t][:], ACTF.Square,
                                     accum_out=sq[:])
                sqs.append(sq)
            for t in range(nT):
                mean = stt.tile([128, 1], f32, tag=f"{pfx}mean{t}")
                nc.vector.tensor_scalar(out=mean[:], in0=sms[t][:],
                                        scalar1=1.0 / DS, scalar2=None,
                                        op0=ALU.mult)
                msq = stt.tile([128, 1], f32, tag="msq")
                nc.vector.tensor_tensor(msq[:], mean[:], mean[:], ALU.mult)
                var = stt.tile([128, 1], f32, tag=f"{pfx}var{t}")
                nc.vector.scalar_tensor_tensor(var[:], sqs[t][:], 1.0 / DS, msq[:],
                                               ALU.mult, ALU.subtract)
                stds.append((mean, var))
            for t in range(nT):
                mean, var = stds[t]
                std = stt.tile([128, 1], f32, tag=f"{pfx}std{t}")
                nc.scalar.activation(std[:], var[:], ACTF.Sqrt, bias=eps_b[:])
                rstd = stt.tile([128, 1], f32, tag=f"{pfx}rstd{t}")
                nc.vector.reciprocal(rstd[:], std[:])
                rstds.append(rstd)
                nmr = stt.tile([128, 1], f32, tag=f"{pfx}nmr{t}")
                nc.vector.scalar_tensor_tensor(nmr[:], mean[:], -1.0, rstd[:],
                                               ALU.mult, ALU.mult)
                nmrs.append(nmr)
            for t in range(nT):
                lnt = xs_pool.tile([128, DS], f16, tag=f"ln{t}")
                nc.vector.scalar_tensor_tensor(
                    lnt[:], xts[t][:], rstds[t][:],
                    nmrs[t][:].broadcast_to([128, DS]),
                    ALU.mult, ALU.add)
                lnts.append(lnt)
            return lnts

        def modulate(lhs_fn, gamma16, gbias16, beta16, lnx16):
            """a = sigmoid(sn@gamma + gb) * lnx + sn@beta; lhs_fn(kt)->snT ktile AP"""
            pg = [ps_mm.tile([128, 384], f32, tag="mm384", name=f"pg{_i}") for _i in range(2)]
            pb = [ps_mm.tile([128, 384], f32, tag="mm384", name=f"pb{_i}") for _i in range(2)]
            for kt in range(KT):
                lhs = lhs_fn(kt)
                for c in range(2):
                    nc.tensor.matmul(
                        pg[c][:], lhs,
                        gamma16[:, kt * DS + c * 384:kt * DS + (c + 1) * 384],
                        start=(kt == 0), stop=False)
                    nc.tensor.matmul(
                        pb[c][:], lhs,
                        beta16[:, kt * DS + c * 384:kt * DS + (c + 1) * 384],
                        start=(kt == 0), stop=(kt == KT - 1))
            for c in range(2):
                nc.tensor.matmul(pg[c][:], ones_row[:],
                                 gbias16[:, c * 384:(c + 1) * 384],
                                 start=False, stop=True)
            sig = scr.tile([128, DS], f16, tag="sig")
            for c in range(2):
                nc.scalar.activation(sig[:, c * 384:(c + 1) * 384], pg[c][:],
                                     ACTF.Sigmoid)
            tmp = scr.tile([128, DS], f16, tag="sigl")
            nc.vector.tensor_tensor(tmp[:], sig[:], lnx16[:], ALU.mult)
            a16 = str16.tile([128, DS], f16, tag="am")
            for c in range(2):
                nc.vector.scalar_tensor_tensor(
                    a16[:, c * 384:(c + 1) * 384],
                    tmp[:, c * 384:(c + 1) * 384], 1.0, pb[c][:],
                    ALU.mult, ALU.add)
            return a16

        def gate_from(sT_sb, gatew16, gbias16, dst16):
            p = [ps_mm.tile([128, 384], f32, tag="mm384", name=f"pgate{_i}") for _i in range(2)]
            for kt in range(KT):
                lhs = sT_sb[:, kt * 128:(kt + 1) * 128]
                for c in range(2):
                    nc.tensor.matmul(
                        p[c][:], lhs,
                        gatew16[:, kt * DS + c * 384:kt * DS + (c + 1) * 384],
                        start=(kt == 0), stop=False)
            for c in range(2):
                nc.tensor.matmul(p[c][:], ones_row[:],
                                 gbias16[:, c * 384:(c + 1) * 384],
                                 start=False, stop=True)
            for c in range(2):
                nc.scalar.activation(dst16[:, c * 384:(c + 1) * 384], p[c][:],
                                     ACTF.Sigmoid)

        # ================= phase 1: LN + modulation =================
        x3 = x_full.rearrange("(t p) d -> p t d", p=128)
        s3 = s_full.rearrange("(t p) d -> p t d", p=128)

        with tc.tile_pool(name="wmod", bufs=1) as wmod, \
             tc.tile_pool(name="xs", bufs=1) as xs:
            agw16 = load_w(wmod, agw, KT, DS, "agw")
            abw16 = load_w(wmod, abw, KT, DS, "abw")

            # staged LN for s (8 full tiles + local)
            s_srcs = [s3[:, t, :] for t in range(JT)] + [s_loc[:, :]]
            lns_all = ln_staged(xs, s_srcs, "s")
            for t in range(JT):
                transpose_768(lns_all[t], snT, [kt * N + t * 128 for kt in range(KT)])
            transpose_768(lns_all[JT], snTl, [kt * 128 for kt in range(KT)])
            # raw s transposed (for gates)
            s16l = str16.tile([128, DS], f16, tag="ln")
            nc.gpsimd.dma_start(s16l[:], s_loc[:, :])
            transpose_768(s16l, sTl, [kt * 128 for kt in range(KT)])

            # staged LN for x (8 full tiles + local; x_loc is f32, dma converts)
            x_srcs = [x3[:, t, :] for t in range(JT)] + [x_loc[:, :]]
            lnx_all = ln_staged(xs, x_srcs, "x")
            nc.vector.tensor_copy(ln_xl[:], lnx_all[JT][:])

            # modulate stretch: scalar does only sigmoids here
            for t in range(JT):
                a16 = modulate(
                    lambda kt: snT[:, kt * N + t * 128:kt * N + (t + 1) * 128],
                    agw16, agb16, abw16, lnx_all[t])
                transpose_768(a16, aT, [kt * N + t * 128 for kt in range(KT)])
            al16 = modulate(
                lambda kt: snTl[:, kt * 128:(kt + 1) * 128],
                agw16, agb16, abw16, ln_xl)
            aTl = pers.tile([128, KT * 128], f16)
            transpose_768(al16, aTl, [kt * 128 for kt in range(KT)])

        ctxsn.close()  # snT no longer needed

        def emit_debug(src_t):
            w = min(src_t.shape[-1], DS)
            dbg = scr.tile([128, DS], f32, tag="dbg")
            nc.vector.memset(dbg[:], 0.0)
            nc.vector.tensor_copy(dbg[:, 0:w], src_t[:, 0:w])
            nc.sync.dma_start(out_ap[:, :], dbg[:])

        if PHASE == 1:
            emit_debug(al16)
            ctx2.close()

        if PHASE >= 2:
            # ====== merged phase 2 + pass A ======
            # k/v projections (tensor-heavy) are interleaved with the
            # pair-bias pass (DMA/vector-heavy) so the tensor work hides
            # under the pair stream instead of running as a serial phase.
            ctx3 = ExitStack()
            pairp = ctx3.enter_context(tc.tile_pool(name="pairp", bufs=2))
            sqp = ctx3.enter_context(tc.tile_pool(name="sqp", bufs=2))
            bstg = ctx3.enter_context(tc.tile_pool(name="bstg", bufs=2))
            pstat = ctx3.enter_context(tc.tile_pool(name="pstat", bufs=1))
            ps_bias = ctx3.enter_context(
                tc.tile_pool(name="ps_bias", bufs=2, space="PSUM"))
            QI = 32
            ones1 = wbc_aug[:, H:H + 1]
            wo16 = load_w(wlate, wo, KT, DS, "wo")
            pair_tiles = {}

            def fetch_pair(c):
                Tt = pairp.tile([128, NLOC * 64], f16, tag="pairT",
                                name=f"pt{c}")
                eng = nc.sync if c % 2 == 0 else nc.scalar
                eng.dma_start(
                    Tt[:], pairT[:, c * NLOC * 64:(c + 1) * NLOC * 64])
                pair_tiles[c] = Tt

            fetch_pair(0)
            fetch_pair(1)

            jts = {}

            def passA_chunk(c):
                jt, ih = divmod(c, 2)
                if ih == 0:
                    jts['b'] = bstg.tile([128, NLOC * CB], f16, tag="bstg",
                                         name=f"bjt{jt}")
                    jts['s'] = bstg.tile([128, 2 * NLOC], f32, tag="sstg",
                                         name=f"sjt{jt}")
                bias_jt, stat_jt = jts['b'], jts['s']
                Tt = pair_tiles[c]
                Tv = Tt[:].rearrange("p (i j) -> p i j", j=128)
                SQv = None
                for hb in range(NLOC // 32):
                    if hb % (QI // 16) == 0:
                        q0 = hb * 16
                        SQ = sqp.tile([128, QI * 128], f16, tag="sq")
                        nc.vector.tensor_tensor(
                            SQ[:],
                            Tt[:, q0 * 128:(q0 + QI) * 128],
                            Tt[:, q0 * 128:(q0 + QI) * 128], ALU.mult)
                        SQv = SQ[:].rearrange("p (i j) -> p i j", j=128)
                    pbias = ps_bias.tile([128, 16 * H + 33], f32, tag="pb")
                    for bi in range(16):
                        i = hb * 16 + bi
                        nc.tensor.matmul(
                            pbias[:, bi * H:(bi + 1) * H],
                            Tv[:, i, :], wbc_aug[:, 0:H],
                            start=True, stop=True, skip_group_check=True)
                        nc.tensor.matmul(
                            pbias[:, 16 * H + bi:16 * H + bi + 1],
                            Tv[:, i, :], ones1,
                            start=True, stop=True, skip_group_check=True)
                        nc.tensor.matmul(
                            pbias[:, 16 * H + 16 + bi:16 * H + 17 + bi],
                            SQv[:, i - q0, :], ones1,
                            start=True, stop=True, skip_group_check=True)
                    i0 = ih * 64 + hb * 16
                    nc.scalar.activation(
                        bias_jt[:, i0 * CB:(i0 + 16) * CB],
                        pbias[:, 0:16 * H], ACTF.Copy)
                    nc.scalar.activation(
                        stat_jt[:, i0:i0 + 16],
                        pbias[:, 16 * H:16 * H + 16], ACTF.Copy)
                    nc.scalar.activation(
                        stat_jt[:, NLOC + i0:NLOC + i0 + 16],
                        pbias[:, 16 * H + 16:16 * H + 32], ACTF.Copy)
                if c + 2 < JT * 2:
                    fetch_pair(c + 2)
                if ih == 1:
                    # per-jt epilogue: LN stats + scaling + DRAM spill
                    mean = pstat.tile([128, NLOC], f32, tag="mean")
                    nc.vector.tensor_scalar(
                        out=mean[:], in0=stat_jt[:, 0:NLOC],
                        scalar1=1.0 / DP, scalar2=None, op0=ALU.mult)
                    msq = pstat.tile([128, NLOC], f32, tag="msq")
                    nc.vector.tensor_tensor(msq[:], mean[:], mean[:], ALU.mult)
                    var = pstat.tile([128, NLOC], f32, tag="var")
                    nc.vector.scalar_tensor_tensor(
                        var[:], stat_jt[:, NLOC:2 * NLOC], 1.0 / DP,
                        msq[:], ALU.mult, ALU.subtract)
                    std = pstat.tile([128, NLOC], f32, tag="std")
                    nc.scalar.activation(std[:], var[:], ACTF.Sqrt,
                                         bias=eps_b[:])
                    rstd = pstat.tile([128, NLOC], f32, tag="rstd")
                    nc.vector.reciprocal(rstd[:], std[:])
                    nc.vector.scalar_tensor_tensor(
                        bias_jt[:].rearrange("p (i c) -> p i c", c=CB),
                        bias_jt[:].rearrange("p (i c) -> p i c", c=CB),
                        1.0,
                        rstd[:, :, None].broadcast_to([128, NLOC, CB]),
                        ALU.mult, ALU.mult)
                    nc.gpsimd.dma_start(bias_dram[jt][:], bias_jt[:])

            with tc.tile_pool(name="wkp", bufs=1) as wkp:
                wk16 = load_w(wkp, wk, KT, DS, "wk")
                for c in range(JT):
                    for u in range(4 * c, 4 * c + 4):
                        h, half = divmod(u, 2)
                        pk = ps_big.tile([128, 512], f32, tag="big")
                        for kt in range(KT):
                            rhs = aT[:, kt * N + half * 512:kt * N + (half + 1) * 512]
                            nc.tensor.matmul(
                                pk[0:DH, :],
                                wk16[:, kt * DS + h * DH:kt * DS + (h + 1) * DH],
                                rhs, start=(kt == 0), stop=(kt == KT - 1))
                        nc.scalar.activation(
                            kT[0:DH, h * N + half * 512:h * N + (half + 1) * 512],
                            pk[0:DH, :], ACTF.Copy)
                    passA_chunk(c)

            with tc.tile_pool(name="wvp", bufs=1) as wvp:
                wv16 = load_w(wvp, wv, KT, DS, "wv")
                for c in range(JT, 2 * JT):
                    t = c - JT
                    pv = [ps_mm.tile([128, 384], f32, tag="mm384", name=f"pv{_i}") for _i in range(2)]
                    for kt in range(KT):
                        lhs = aT[:, kt * N + t * 128:kt * N + (t + 1) * 128]
                        for cc in range(2):
                            nc.tensor.matmul(
                                pv[cc][:], lhs,
                                wv16[:, kt * DS + cc * 384:kt * DS + (cc + 1) * 384],
                                start=(kt == 0), stop=(kt == KT - 1))
                    for cc in range(2):
                        dst = v_sb[:, t * H * VH + cc * 8 * VH:
                                   t * H * VH + (cc + 1) * 8 * VH]
                        nc.scalar.activation(
                            dst.rearrange("p (h v) -> p h v", v=VH)[:, :, 0:DH],
                            pv[cc][:].rearrange("p (h d) -> p h d", d=DH),
                            ACTF.Copy, scale=mask_sb[:, t:t + 1])
                    mdst = v_sb[:, t * H * VH:(t + 1) * H * VH]
                    nc.vector.tensor_copy(
                        mdst.rearrange("p (h v) -> p h v", v=VH)[:, :, DH:VH],
                        mask_sb[:, t:t + 1, None].broadcast_to([128, H, 1]))
                    passA_chunk(c)

            ctx3.close()  # free pair + SQ + staging SBUF space
            ctx2.close()  # free aT SBUF space

            # qT local (scaled by 1/sqrt(dh), + bq): per head, base 0
            with tc.tile_pool(name="wqp", bufs=1) as wqp:
                wq16 = load_w(wqp, wq, KT, DS, "wq")
                for hq in range(0, H, 4):
                    pq = ps_big.tile([128, 512], f32, tag="big")
                    for hh in range(4):
                        h = hq + hh
                        for kt in range(KT):
                            rhs = aTl[:, kt * 128:(kt + 1) * 128]
                            nc.tensor.matmul(
                                pq[0:DH, hh * 128:(hh + 1) * 128],
                                wq16[:, kt * DS + h * DH:kt * DS + (h + 1) * DH],
                                rhs, start=(kt == 0 and hh == 0),
                                stop=(kt == KT - 1 and hh == 3),
                                skip_group_check=True)
                        nc.scalar.activation(
                            qT[0:DH, h * 128:(h + 1) * 128],
                            pq[0:DH, hh * 128:(hh + 1) * 128],
                            ACTF.Identity, bias=bqs[0:DH, h:h + 1], scale=SCALE)

            with tc.tile_pool(name="wgg", bufs=1) as wgg:
                wg16 = load_w(wgg, wg, KT, DS, "wg")
                atgw16 = load_w(wgg, atgw, KT, DS, "atgw")
                ffgw16 = load_w(wgg, ffgw, KT, DS, "ffgw")

                # g = sigmoid(a_loc @ wg)
                pgt = [ps_mm.tile([128, 384], f32, tag="mm384", name=f"pgt{_i}") for _i in range(2)]
                for kt in range(KT):
                    lhs = aTl[:, kt * 128:(kt + 1) * 128]
                    for cc in range(2):
                        nc.tensor.matmul(
                            pgt[cc][:], lhs,
                            wg16[:, kt * DS + cc * 384:kt * DS + (cc + 1) * 384],
                            start=(kt == 0), stop=(kt == KT - 1))
                for cc in range(2):
                    nc.scalar.activation(g16[:, cc * 384:(cc + 1) * 384], pgt[cc][:],
                                         ACTF.Sigmoid)
                gate_from(sTl, atgw16, atgb16, gate_a)
                gate_from(sTl, ffgw16, ffgb16, gate_f)
            if PHASE == 2:
                emit_debug(g16)

            # ================= phase 3: attention =================
            if PHASE >= 3:

                # prefetch ff weights during pass B (gpsimd queue)
                wff = ctx.enter_context(tc.tile_pool(name="wff", bufs=1))
                fgw16 = load_w(wff, fgw, KT, DS, "fgw")
                fbw16 = load_w(wff, fbw, KT, DS, "fbw")
                fw116 = load_w(wff, fw1, KT, DFF, "fw1", eng=nc.sync)
                fw216 = load_w(wff, fw2, KT, DFF, "fw2", eng=nc.sync)

                with tc.tile_pool(name="biasrd", bufs=2) as biasrd, \
                     tc.tile_pool(name="attnT", bufs=3) as attnTp, \
                     tc.tile_pool(name="ps_o", bufs=1, space="PSUM") as ps_o:

                    po = [ps_o.tile([128, 8 * VH], f32, tag=f"o{i}", name=f"po{i}") for i in range(2)]

                    # ---- pass B: attention over all j-tiles ----
                    for jt in range(JT):
                        brt = biasrd.tile([128, NLOC * CB], f16, tag="brd")
                        nc.sync.dma_start(brt[:], bias_dram[jt][:])
                        bias_hi = brt[:].rearrange("p (i h) -> p h i", h=CB)
                        for grp in range(4):
                            pl = ps_big.tile([128, 512], f32, tag="big")
                            for hh in range(4):
                                h = grp * 4 + hh
                                nc.tensor.matmul(
                                    pl[:, hh * 128:(hh + 1) * 128],
                                    kT[0:DH,
                                       h * N + jt * 128:h * N + (jt + 1) * 128],
                                    qT[0:DH, h * 128:(h + 1) * 128],
                                    start=(hh == 0), stop=False,
                                    skip_group_check=True)
                            nc.tensor.matmul(
                                pl[:], ident[:],
                                bias_hi[:, grp * 4:(grp + 1) * 4, :],
                                start=False, stop=True, skip_group_check=True)
                            at16 = attnTp.tile([128, 512], f16, tag="attnT")
                            nc.scalar.activation(at16[:], pl[:], ACTF.Exp)
                            for hh in range(4):
                                h = grp * 4 + hh
                                ho = h % 8
                                pot = po[h // 8]
                                lhs = at16[:, hh * 128:(hh + 1) * 128]
                                nc.tensor.matmul(
                                    pot[:, ho * VH:(ho + 1) * VH], lhs,
                                    v_sb[:, jt * H * VH + h * VH:
                                         jt * H * VH + (h + 1) * VH],
                                    start=(jt == 0 and ho == 0),
                                    stop=(jt == JT - 1 and ho == 7),
                                    skip_group_check=True)

                    if KSUB == 0:
                        # ---- normalize + gate + output projection ----
                        rd = stt.tile([128, H], f32, tag="rd")
                        for i in range(2):
                            den = po[i][:].rearrange("p (h d) -> p h d", d=VH)[:, :, DH]
                            nc.vector.reciprocal(rd[:, i * 8:(i + 1) * 8], den)
                        g2 = scr.tile([128, DS], f16, tag="sc16a")
                        nc.vector.tensor_tensor(
                            g2[:].rearrange("p (h d) -> p h d", d=DH),
                            g16[:].rearrange("p (h d) -> p h d", d=DH),
                            rd[:, :, None].broadcast_to([128, H, DH]), ALU.mult)
                        go = str16.tile([128, DS], f16, tag="go")
                        for i in range(2):
                            nc.vector.scalar_tensor_tensor(
                                go[:, i * 384:(i + 1) * 384].rearrange(
                                    "p (h d) -> p h d", d=DH),
                                po[i][:].rearrange("p (h d) -> p h d", d=VH)[:, :, 0:DH],
                                1.0,
                                g2[:, i * 384:(i + 1) * 384].rearrange(
                                    "p (h d) -> p h d", d=DH),
                                ALU.mult, ALU.mult)
                        goT = pers.tile([128, KT * 128], f16)
                        transpose_768(go, goT, [kt * 128 for kt in range(KT)])
                        if PHASE == 3:
                            emit_debug(go)

                if PHASE >= 4 and KSUB == 0:
                    scr4 = ctx.enter_context(tc.tile_pool(name="scr4", bufs=2))
                    pao = [ps_mm.tile([128, 384], f32, tag="mm384", name=f"pao{_i}") for _i in range(2)]
                    for kt in range(KT):
                        lhs = goT[:, kt * 128:(kt + 1) * 128]
                        for c in range(2):
                            nc.tensor.matmul(
                                pao[c][:], lhs,
                                wo16[:, kt * DS + c * 384:kt * DS + (c + 1) * 384],
                                start=(kt == 0), stop=(kt == KT - 1))
                    # y_part = x_loc + gate_a * attn_out
                    xl32 = scr4.tile([128, DS], f32, tag="s32")
                    nc.sync.dma_start(xl32[:], x_loc[:, :])
                    for c in range(2):
                        nc.vector.scalar_tensor_tensor(
                            y_part[:, c * 384:(c + 1) * 384],
                            gate_a[:, c * 384:(c + 1) * 384], 1.0, pao[c][:],
                            ALU.mult, ALU.mult)
                    nc.vector.tensor_tensor(y_part[:], y_part[:], xl32[:], ALU.add)

                    # ================= phase 4: feed-forward =================
                    # load fw3 now (gpsimd queue; needed only after m16/mT)
                    fw316 = load_w(wff, fw3, FKT, DS, "fw3")
                    f16l = modulate(
                        lambda kt: snTl[:, kt * 128:(kt + 1) * 128],
                        fgw16, fgb16, fbw16, ln_xl)
                    fT = pers.tile([128, KT * 128], f16)
                    transpose_768(f16l, fT, [kt * 128 for kt in range(KT)])

                    m16 = str16.tile([128, DFF], f16, tag="m16")
                    for c in range(4):  # 4 chunks of 384 over DFF
                        ph1 = ps_mm.tile([128, 384], f32, tag="mm384")
                        ph2 = ps_mm.tile([128, 384], f32, tag="mm384")
                        for kt in range(KT):
                            lhs = fT[:, kt * 128:(kt + 1) * 128]
                            nc.tensor.matmul(
                                ph1[:], lhs,
                                fw116[:, kt * DFF + c * 384:kt * DFF + (c + 1) * 384],
                                start=(kt == 0), stop=(kt == KT - 1))
                            nc.tensor.matmul(
                                ph2[:], lhs,
                                fw216[:, kt * DFF + c * 384:kt * DFF + (c + 1) * 384],
                                start=(kt == 0), stop=(kt == KT - 1))
                        sg = scr4.tile([128, 384], f16, tag="sil")
                        nc.scalar.activation(sg[:], ph1[:], ACTF.Sigmoid)
                        pp = scr4.tile([128, 384], f16, tag="pp")
                        nc.vector.scalar_tensor_tensor(
                            pp[:], sg[:], 1.0, ph1[:], ALU.mult, ALU.mult)
                        nc.vector.scalar_tensor_tensor(
                            m16[:, c * 384:(c + 1) * 384], pp[:], 1.0,
                            ph2[:], ALU.mult, ALU.mult)
                    mT = pers.tile([128, FKT * 128], f16)
                    transpose_768(m16, mT, [kt * 128 for kt in range(FKT)])

                    pff = [ps_mm.tile([128, 384], f32, tag="mm384", name=f"pff{_i}") for _i in range(2)]
                    for kt in range(FKT):
                        lhs = mT[:, kt * 128:(kt + 1) * 128]
                        for c in range(2):
                            nc.tensor.matmul(
                                pff[c][:], lhs,
                                fw316[:, kt * DS + c * 384:kt * DS + (c + 1) * 384],
                                start=(kt == 0), stop=(kt == FKT - 1))
                    yout = scr4.tile([128, DS], f32, tag="s32")
                    for c in range(2):
                        nc.vector.scalar_tensor_tensor(
                            yout[:, c * 384:(c + 1) * 384],
                            gate_f[:, c * 384:(c + 1) * 384], 1.0, pff[c][:],
                            ALU.mult, ALU.mult)
                    nc.vector.tensor_tensor(yout[:], yout[:], y_part[:], ALU.add)
                    nc.sync.dma_start(out_ap[:, :], yout[:])

    nc.compile()
    return nc


def _get_nc():
    if "nc" not in _CACHE:
        _CACHE["nc"] = _build()
    return _CACHE["nc"]


def prepare_in_maps(inputs):
    x = np.asarray(inputs["x"], dtype=np.float32).reshape(N, DS)
    s = np.asarray(inputs["single_cond"], dtype=np.float32).reshape(N, DS)
    pc = np.asarray(inputs["pair_cond"], dtype=np.float32).reshape(N, N, DP)
    mask = np.asarray(inputs["mask"]).reshape(N, 1).astype(np.float32)
    x16 = x.astype(np.float16)
    s16 = s.astype(np.float16)
    w16 = {k: np.asarray(inputs[k], dtype=np.float32).astype(np.float16) for k in [
        "attn_gamma_w", "attn_beta_w", "wq", "wk", "wv", "wb", "wg", "wo",
        "attn_gate_w", "ff_gamma_w", "ff_beta_w", "ff_w1", "ff_w2", "ff_w3",
        "ff_gate_w"]}
    rows16 = {k: np.asarray(inputs[k], dtype=np.float32).reshape(1, DS).astype(np.float16)
              for k in ["attn_gamma_b", "attn_gate_b", "ff_gamma_b", "ff_gate_b"]}
    bq_row = np.asarray(inputs["bq"], dtype=np.float32).reshape(1, DS)
    in_maps = []
    for c in range(NC_):
        blk = pc[c * NLOC:(c + 1) * NLOC]          # [i_loc, j, p]
        pairTm = np.empty((DP, JT, NLOC, 128), np.float16)
        bt = blk.transpose(2, 0, 1)                # [p, i_loc, j] view
        for jt in range(JT):
            pairTm[:, jt] = bt[:, :, jt * 128:(jt + 1) * 128]
        m = {"x_full": x16, "s_full": s16,
             "x_loc": x[c * NLOC:(c + 1) * NLOC],
             "s_loc": s16[c * NLOC:(c + 1) * NLOC],
             "pairT": pairTm.reshape(DP, N * NLOC),
             "mask01": mask,
             "bq": bq_row}
        m.update(w16)
        m.update(rows16)
        in_maps.append(m)
    return in_maps


def kernel(**inputs):
    from concourse.bass_utils import run_bass_kernel_spmd
    nc = _get_nc()
    in_maps = prepare_in_maps(inputs)
    _CACHE["in_maps"] = in_maps
    globals()["_last_in_maps"] = in_maps
    res = run_bass_kernel_spmd(nc, in_maps, list(range(NC_)))
    out = np.concatenate([res.results[c]["out"] for c in range(NC_)], axis=0)
    return out.reshape(B, N, DS).astype(np.float32)
